# revision 8
# baseline (speedup 1.0000x reference)
"""RNN-T Joint network kernel for 8x Trainium2 NeuronCores.

logits[b,t,u,v] = enc_out[b,t,:] @ W[v,:512] + pred_out[b,u,:] @ W[v,512:] + b[v]

Sharding: data-parallel over (B=4) x (T split in 2) -> 8 shards.
Core i handles b = i//2, t in [128*(i%2), 128*(i%2)+128).
Each core computes a contiguous (128, 64, 2048) f32 output slab (64 MB),
which makes the store phase HBM-bound at ~358 GB/s -> ~187 us/core floor.

Per-core plan (t on partitions, v on free dim):
  1. Host pre-transposes W/enc/pred to contraction-major and casts to
     bf16 (rel err ~3e-3, tolerance 2e-2), so there are no on-device
     PE transposes and the W load is half the bytes.
  2. GEMMs in bf16 (1 cyc/row on PE), k-outer so accumulation into 8
     PSUM banks overlaps the streaming W k-tile loads.
  3. For each u: broadcast pred_b[u,:] (bf16) to 128 partitions with a
     K=64 onehot matmul into PSUM, DVE-add enc_proj + PSUM -> SBUF,
     then 1 MB DMA store (8 KB contiguous per partition).
"""

import numpy as np

B, T, U = 4, 256, 64
D_ENC, D_PRED, VOCAB = 512, 512, 2048
D = D_ENC + D_PRED
TT = 128  # t rows per core
N_CORES = 8
KE = D_ENC // 128  # 4 k-tiles per GEMM half
NV = VOCAB // 512  # 4 v-chunks of 512

_cache = {}


def _build():
    import concourse.bacc as bacc
    import concourse.mybir as mybir
    from concourse.tile import TileContext

    f32 = mybir.dt.float32
    bf16 = mybir.dt.bfloat16

    nc = bacc.Bacc("TRN2", target_bir_lowering=False, debug=False, num_devices=N_CORES)
    # enc^T (4 k-tiles of [128,128]) | pred^T (4 k-tiles of [128,64]) packed
    # into one [128, 768] tensor -> one big-line DMA instead of 8 tiny ones.
    ep_d = nc.dram_tensor("ep", (128, KE * TT + KE * U), bf16, kind="ExternalInput")
    bias_d = nc.dram_tensor("bias", (1, VOCAB), bf16, kind="ExternalInput")
    w_d = nc.dram_tensor("w", (2 * KE, 128, VOCAB), bf16, kind="ExternalInput")
    onehot_d = nc.dram_tensor("onehot", (U, U * 128), bf16, kind="ExternalInput")
    out_d = nc.dram_tensor("out", (TT, U, VOCAB), f32, kind="ExternalOutput")

    with TileContext(nc) as tc:
        with (
            tc.tile_pool(name="persist", bufs=1) as persist,
            tc.tile_pool(name="outp", bufs=4) as outp,
        ):
            ep = persist.tile([128, KE * TT + KE * U], bf16)
            bias_sb = persist.tile([1, VOCAB], bf16)
            w_t = persist.tile([128, 2 * KE * VOCAB], bf16)
            onehot = persist.tile([U, U * 128], bf16)
            enc_proj = persist.tile([128, VOCAB], f32)
            pred_b = persist.tile([U, VOCAB], bf16)
            enc_t = ep[:, 0:KE * TT]
            pred_t = ep[:, KE * TT:]

            # issue order = arrival order: GEMM operands first, W k-tiles
            # streaming, onehot (first needed by the u-loop) last.
            nc.sync.dma_start(out=ep, in_=ep_d[:])
            nc.sync.dma_start(out=bias_sb, in_=bias_d[:])
            for k in range(2 * KE):
                nc.sync.dma_start(
                    out=w_t[:, k * VOCAB:(k + 1) * VOCAB], in_=w_d[k]
                )
            nc.sync.dma_start(out=onehot, in_=onehot_d[:])

            # ---- GEMMs, k-outer so PE consumes W k-tiles as they land.
            with tc.tile_pool(name="ps_a", bufs=1, space="PSUM") as ps_a:
                for c in range(NV):
                    ps = ps_a.tile([128, 512], f32, name="ps_gemm", bufs=4)
                    for k in range(KE):
                        nc.tensor.matmul(
                            ps,
                            lhsT=enc_t[:, k * TT:(k + 1) * TT],
                            rhs=w_t[:, k * VOCAB + c * 512: k * VOCAB + (c + 1) * 512],
                            start=(k == 0),
                            stop=(k == KE - 1),
                        )
                    nc.scalar.copy(out=enc_proj[:, c * 512:(c + 1) * 512], in_=ps)
                for c in range(NV):
                    ps = ps_a.tile([128, 512], f32, name="ps_gemm", bufs=4)
                    for k in range(KE):
                        kd = KE + k  # W_pred half
                        nc.tensor.matmul(
                            ps[:U],
                            lhsT=pred_t[:, k * U:(k + 1) * U],
                            rhs=w_t[:, kd * VOCAB + c * 512: kd * VOCAB + (c + 1) * 512],
                            start=(k == 0),
                            stop=False,
                        )
                    # onehot row 0 is all-ones over its first 128 cols
                    nc.tensor.matmul(
                        ps[:U],
                        lhsT=onehot[0:1, 0:U],
                        rhs=bias_sb[:, c * 512:(c + 1) * 512],
                        start=False,
                        stop=True,
                    )
                    nc.scalar.copy(out=pred_b[:, c * 512:(c + 1) * 512], in_=ps[:U])

            # ---- main loop: one (128, 2048) output tile per u
            with tc.tile_pool(name="ps_b", bufs=2, space="PSUM") as ps_b:
                for u in range(U):
                    ps = ps_b.tile([128, VOCAB], f32)
                    for c in range(NV):
                        nc.tensor.matmul(
                            ps[:, c * 512:(c + 1) * 512],
                            lhsT=onehot[:, u * 128:(u + 1) * 128],
                            rhs=pred_b[:, c * 512:(c + 1) * 512],
                            start=True,
                            stop=True,
                        )
                    o = outp.tile([128, VOCAB], f32)
                    nc.vector.tensor_add(o, enc_proj, ps)
                    nc.sync.dma_start(out=out_d[:, u, :], in_=o)

    nc.compile()
    return nc


def kernel(enc_out, pred_out, W, b):
    import os

    import ml_dtypes
    from concourse.bass_utils import run_bass_kernel_spmd

    if "nc" not in _cache:
        _cache["nc"] = _build()
    nc = _cache["nc"]
    trace = bool(os.environ.get("KJN_TRACE"))

    bf = ml_dtypes.bfloat16
    # W^T (d-major) bf16 k-tiles: (2*KE, 128, VOCAB)
    w_kt = np.ascontiguousarray(
        W.astype(np.float32).T.astype(bf).reshape(2 * KE, 128, VOCAB)
    )
    bias = np.ascontiguousarray(b.astype(np.float32).astype(bf).reshape(1, VOCAB))
    onehot = np.zeros((U, U * 128), dtype=bf)
    for u in range(U):
        onehot[u, u * 128:(u + 1) * 128] = 1.0

    in_maps = []
    for i in range(N_CORES):
        bi, th = i // 2, i % 2
        # [128 partitions, k*128 + t] / [128, k*64 + u] packed side by side
        enc_kt = (
            enc_out[bi, th * TT:(th + 1) * TT, :]
            .astype(np.float32).T.astype(bf).reshape(KE, 128, TT).transpose(1, 0, 2)
            .reshape(128, KE * TT)
        )
        pred_kt = (
            pred_out[bi].astype(np.float32).T.astype(bf)
            .reshape(KE, 128, U).transpose(1, 0, 2).reshape(128, KE * U)
        )
        ep = np.ascontiguousarray(np.concatenate([enc_kt, pred_kt], axis=1))
        in_maps.append({
            "ep": ep,
            "w": w_kt,
            "bias": bias,
            "onehot": onehot,
        })

    kw = {}
    if trace:
        kw = dict(trace=True, trace_cores=[0], stitch_traces=False)
    res = run_bass_kernel_spmd(nc, in_maps, core_ids=list(range(N_CORES)), **kw)
    if trace:
        print(f"HW exec time: {res.exec_time_ns} ns")
        print(f"trace: {res.instructions_and_trace[1] if res.instructions_and_trace else None}")
        print(f"profile_json: {res.profile_json}")
    out = np.empty((B, T, U, VOCAB), dtype=np.float32)
    for i in range(N_CORES):
        bi, th = i // 2, i % 2
        out[bi, th * TT:(th + 1) * TT] = res.results[i]["out"]
    return out


# revision 11
# speedup vs baseline: 1.2646x; 1.2646x over previous
"""RNN-T Joint network kernel for 8x Trainium2 NeuronCores.

logits[b,t,u,v] = enc_out[b,t,:] @ W[v,:512] + pred_out[b,u,:] @ W[v,512:] + b[v]

Sharding: data-parallel over (B=4) x (T split in 2) -> 8 shards.
Core i handles b = i//2, t in [128*(i%2), 128*(i%2)+128).
Each core computes a contiguous (128, 64, 2048) f32 output slab (64 MB),
which makes the store phase HBM-bound at ~358 GB/s -> ~187 us/core floor.

Per-core plan (t on partitions, v on free dim):
  1. Host pre-transposes W/enc/pred to contraction-major and casts to
     bf16 (rel err ~3e-3, tolerance 2e-2), so there are no on-device
     PE transposes and the W load is half the bytes.
  2. GEMMs in bf16 (1 cyc/row on PE), k-outer so accumulation into 8
     PSUM banks overlaps the streaming W k-tile loads.
  3. For each u: broadcast pred_b[u,:] (bf16) to 128 partitions with a
     K=64 onehot matmul into PSUM, DVE-add enc_proj + PSUM -> SBUF,
     then 1 MB DMA store (8 KB contiguous per partition).
"""

import numpy as np

B, T, U = 4, 256, 64
D_ENC, D_PRED, VOCAB = 512, 512, 2048
D = D_ENC + D_PRED
TT = 128  # t rows per core
N_CORES = 8
KE = D_ENC // 128  # 4 k-tiles per GEMM half
NV = VOCAB // 512  # 4 v-chunks of 512

_cache = {}


def _build():
    import concourse.bacc as bacc
    import concourse.mybir as mybir
    from concourse.tile import TileContext

    f32 = mybir.dt.float32
    bf16 = mybir.dt.bfloat16

    nc = bacc.Bacc("TRN2", target_bir_lowering=False, debug=False, num_devices=N_CORES)
    # enc^T (4 k-tiles of [128,128]) | pred^T (4 k-tiles of [128,64]) | a
    # 64-wide ones row (bias matmul lhsT) packed into one [128, 832] tensor
    # -> one big-line DMA instead of 9 tiny ones.
    ep_d = nc.dram_tensor("ep", (128, KE * TT + KE * U + U), bf16, kind="ExternalInput")
    bias_d = nc.dram_tensor("bias", (1, VOCAB), bf16, kind="ExternalInput")
    w_d = nc.dram_tensor("w", (2 * KE, 128, VOCAB), bf16, kind="ExternalInput")
    onehot_d = nc.dram_tensor("onehot", (U, U * 128), bf16, kind="ExternalInput")
    out_d = nc.dram_tensor("out", (TT, U, VOCAB), f32, kind="ExternalOutput")

    with TileContext(nc) as tc:
        with (
            tc.tile_pool(name="persist", bufs=1) as persist,
            tc.tile_pool(name="outp", bufs=4) as outp,
        ):
            ep = persist.tile([128, KE * TT + KE * U + U], bf16)
            bias_sb = persist.tile([1, VOCAB], bf16)
            w_t = persist.tile([128, 2 * KE * VOCAB], bf16)
            onehot = persist.tile([U, U * 128], bf16)
            enc_proj = persist.tile([128, VOCAB], f32)
            pred_b = persist.tile([U, VOCAB], bf16)
            enc_t = ep[:, 0:KE * TT]
            pred_t = ep[:, KE * TT:KE * (TT + U)]
            ones_row = ep[0:1, KE * (TT + U):]

            # issue order = arrival order (loads drain one queue serially):
            # small GEMM operands + onehot first, then the W_pred half, then
            # the W_enc half; the enc path (GEMM -> copy -> DVE add) is the
            # longest post-arrival chain so its W half arriving last is fine.
            nc.sync.dma_start(out=ep, in_=ep_d[:])
            nc.sync.dma_start(out=bias_sb, in_=bias_d[:])
            nc.sync.dma_start(out=onehot, in_=onehot_d[:])
            for k in range(KE, 2 * KE):
                nc.sync.dma_start(
                    out=w_t[:, k * VOCAB:(k + 1) * VOCAB], in_=w_d[k]
                )
            for k in range(KE):
                nc.sync.dma_start(
                    out=w_t[:, k * VOCAB:(k + 1) * VOCAB], in_=w_d[k]
                )

            # ---- GEMMs, k-outer so PE consumes W k-tiles as they land.
            with tc.tile_pool(name="ps_a", bufs=1, space="PSUM") as ps_a:
                for c in range(NV):
                    ps = ps_a.tile([128, 512], f32, name="ps_gemm", bufs=4)
                    for k in range(KE):
                        kd = KE + k  # W_pred half
                        nc.tensor.matmul(
                            ps[:U],
                            lhsT=pred_t[:, k * U:(k + 1) * U],
                            rhs=w_t[:, kd * VOCAB + c * 512: kd * VOCAB + (c + 1) * 512],
                            start=(k == 0),
                            stop=False,
                        )
                    # bias via K=1 ones matmul against the ep ones row
                    nc.tensor.matmul(
                        ps[:U],
                        lhsT=ones_row,
                        rhs=bias_sb[:, c * 512:(c + 1) * 512],
                        start=False,
                        stop=True,
                    )
                    nc.scalar.copy(out=pred_b[:, c * 512:(c + 1) * 512], in_=ps[:U])
                for c in range(NV):
                    ps = ps_a.tile([128, 512], f32, name="ps_gemm", bufs=4)
                    for k in range(KE):
                        nc.tensor.matmul(
                            ps,
                            lhsT=enc_t[:, k * TT:(k + 1) * TT],
                            rhs=w_t[:, k * VOCAB + c * 512: k * VOCAB + (c + 1) * 512],
                            start=(k == 0),
                            stop=(k == KE - 1),
                        )
                    nc.scalar.copy(out=enc_proj[:, c * 512:(c + 1) * 512], in_=ps)

            # ---- main loop: one (128, 2048) output tile per u
            with tc.tile_pool(name="ps_b", bufs=2, space="PSUM") as ps_b:
                for u in range(U):
                    ps = ps_b.tile([128, VOCAB], f32)
                    for c in range(NV):
                        nc.tensor.matmul(
                            ps[:, c * 512:(c + 1) * 512],
                            lhsT=onehot[:, u * 128:(u + 1) * 128],
                            rhs=pred_b[:, c * 512:(c + 1) * 512],
                            start=True,
                            stop=True,
                        )
                    o = outp.tile([128, VOCAB], f32)
                    nc.vector.tensor_add(o, enc_proj, ps)
                    nc.sync.dma_start(out=out_d[:, u, :], in_=o)

    nc.compile()
    return nc


def kernel(enc_out, pred_out, W, b):
    import os

    import ml_dtypes
    from concourse.bass_utils import run_bass_kernel_spmd

    if "nc" not in _cache:
        _cache["nc"] = _build()
    nc = _cache["nc"]
    trace = bool(os.environ.get("KJN_TRACE"))

    bf = ml_dtypes.bfloat16
    # W^T (d-major) bf16 k-tiles: (2*KE, 128, VOCAB)
    w_kt = np.ascontiguousarray(
        W.astype(np.float32).T.astype(bf).reshape(2 * KE, 128, VOCAB)
    )
    bias = np.ascontiguousarray(b.astype(np.float32).astype(bf).reshape(1, VOCAB))
    onehot = np.zeros((U, U * 128), dtype=bf)
    for u in range(U):
        onehot[u, u * 128:(u + 1) * 128] = 1.0

    in_maps = []
    for i in range(N_CORES):
        bi, th = i // 2, i % 2
        # [128 partitions, k*128 + t] / [128, k*64 + u] packed side by side
        enc_kt = (
            enc_out[bi, th * TT:(th + 1) * TT, :]
            .astype(np.float32).T.astype(bf).reshape(KE, 128, TT).transpose(1, 0, 2)
            .reshape(128, KE * TT)
        )
        pred_kt = (
            pred_out[bi].astype(np.float32).T.astype(bf)
            .reshape(KE, 128, U).transpose(1, 0, 2).reshape(128, KE * U)
        )
        ones_col = np.zeros((128, U), dtype=bf)
        ones_col[0, :] = 1.0
        ep = np.ascontiguousarray(np.concatenate([enc_kt, pred_kt, ones_col], axis=1))
        in_maps.append({
            "ep": ep,
            "w": w_kt,
            "bias": bias,
            "onehot": onehot,
        })

    kw = {}
    if trace:
        kw = dict(trace=True, trace_cores=[0], stitch_traces=False)
    res = run_bass_kernel_spmd(nc, in_maps, core_ids=list(range(N_CORES)), **kw)
    if trace:
        print(f"HW exec time: {res.exec_time_ns} ns")
        print(f"trace: {res.instructions_and_trace[1] if res.instructions_and_trace else None}")
        print(f"profile_json: {res.profile_json}")
    out = np.empty((B, T, U, VOCAB), dtype=np.float32)
    for i in range(N_CORES):
        bi, th = i // 2, i % 2
        out[bi, th * TT:(th + 1) * TT] = res.results[i]["out"]
    return out
